# revision 10
# baseline (speedup 1.0000x reference)
"""Trainium2 Bass kernel for nn_AttentionBlock (B=16, S=1024, H=256, NH=8).

Strategy: data-parallel over batch — each of 8 NeuronCores handles 2 batches
end-to-end (no collectives). Per core:
  - project q,k,v -> Q^T,K^T (bf16, [out_feat, token] layout) and V (natural)
  - energy = Q_h K_h^T / sqrt(hd) + bias  via PE matmuls; bias is injected
    into PSUM with an identity-weight matmul so the QK matmul accumulates
    on top of it
  - softmax without max-subtraction (logits are ~N(0, 0.15), safe):
    exp on ScalarE with fused row-sum (accum_out), reciprocal + per-row
    scale on VectorE
  - attention matrix (the big 512 MB output) is DMAed out tile by tile
  - the MLP tail only needs x.sum(axis=1) = colsum(attention) @ V, so the
    full attention@V einsum is never materialized: colsums come from a
    ones/recip-weighted PE matmul over the unnormalized exp tiles
  - tail: xs@W1.T + BN(eval) + Mish + residual, all tiny, on-chip
"""

import os
import sys

sys.path.insert(0, "/opt/trn_rl_repo")

import numpy as np

import concourse.bass as bass
import concourse.bacc as bacc
import concourse.mybir as mybir
import concourse.tile as tile
from concourse.bass_utils import run_bass_kernel_spmd

N_CORES = 8
B, SQ, SK, H, NH = 16, 1024, 1024, 256, 8
HD = H // NH  # 32
BL = B // N_CORES  # 2 batches per core
T = BL * SQ  # 2048 local tokens
NTC = T // 128  # 16 token chunks
INV_SQRT_HD = float(1.0 / np.sqrt(HD))
BN_EPS = 1e-5

F32 = mybir.dt.float32
BF16 = mybir.dt.bfloat16
AF = mybir.ActivationFunctionType


def build():
    nc = bacc.Bacc()

    q_d = nc.dram_tensor("q", [T, H], F32, kind="ExternalInput")
    k_d = nc.dram_tensor("k", [T, H], BF16, kind="ExternalInput")
    v_d = nc.dram_tensor("v", [T, H], BF16, kind="ExternalInput")
    bias_d = nc.dram_tensor("bias", [NH, SQ, SK], BF16, kind="ExternalInput")
    wq_d = nc.dram_tensor("wq", [H, H], F32, kind="ExternalInput")
    wk_d = nc.dram_tensor("wk", [H, H], F32, kind="ExternalInput")
    wv_d = nc.dram_tensor("wv", [H, H], F32, kind="ExternalInput")
    w1_d = nc.dram_tensor("w1", [H, H], F32, kind="ExternalInput")
    b1_d = nc.dram_tensor("b1", [H], F32, kind="ExternalInput")
    gamma_d = nc.dram_tensor("gamma", [H], F32, kind="ExternalInput")
    beta_d = nc.dram_tensor("beta", [H], F32, kind="ExternalInput")
    rmean_d = nc.dram_tensor("rmean", [H], F32, kind="ExternalInput")
    rvar_d = nc.dram_tensor("rvar", [H], F32, kind="ExternalInput")
    ident_d = nc.dram_tensor("ident", [128, 128], F32, kind="ExternalInput")

    attn_d = nc.dram_tensor("attn", [BL, NH, SQ, SK], F32, kind="ExternalOutput")
    out_d = nc.dram_tensor("out", [BL, H], F32, kind="ExternalOutput")

    mm = nc.tensor

    with tile.TileContext(nc) as tc:
        with tc.tile_pool(name="persist", bufs=1) as persist:
            # --- constants ---
            id_f32 = persist.tile([128, 128], F32, tag="idf")
            nc.sync.dma_start(id_f32, ident_d[:])
            id_bf16 = persist.tile([128, 128], BF16, tag="idb")
            nc.scalar.activation(id_bf16, id_f32, AF.Copy)

            # persistent activations
            QT = [persist.tile([128, T], BF16, tag=f"qt{m}", name=f"qt{m}") for m in range(2)]
            KT = [persist.tile([128, T], BF16, tag=f"kt{m}", name=f"kt{m}") for m in range(2)]
            Vn = persist.tile([128, NTC * H], BF16, tag="vn")
            WT = {
                w: [persist.tile([128, H], BF16, tag=f"w{w}{ic}", name=f"w{w}{ic}") for ic in range(2)]
                for w in ("q", "k", "v", "1")
            }
            resT = [persist.tile([128, BL], F32, tag=f"res{ic}", name=f"res{ic}") for ic in range(2)]
            S_all = persist.tile([BL * NH, SK], F32, tag="sall")
            sT_all = persist.tile([128, NTC * BL * NH // 2], BF16, tag="stall")
            scaleA = persist.tile([128, 2], F32, tag="scla")
            biasB = persist.tile([128, 2], F32, tag="bnb")

            # ---------------- prologue ----------------
            with tc.tile_pool(name="pro", bufs=2) as pro, \
                 tc.tile_pool(name="props", bufs=4, space="PSUM") as props:
                # natural-layout loads
                q_n = pro.tile([128, NTC * H], F32, tag="qn", bufs=1)
                nc.sync.dma_start(
                    q_n.rearrange("p (tc i) -> p tc i", tc=NTC),
                    q_d[:].rearrange("(tc p) i -> p tc i", p=128),
                )
                k_n = pro.tile([128, NTC * H], BF16, tag="kn", bufs=1)
                nc.sync.dma_start(
                    k_n.rearrange("p (tc i) -> p tc i", tc=NTC),
                    k_d[:].rearrange("(tc p) i -> p tc i", p=128),
                )
                v_n = pro.tile([128, NTC * H], BF16, tag="vnat", bufs=1)
                nc.sync.dma_start(
                    v_n.rearrange("p (tc i) -> p tc i", tc=NTC),
                    v_d[:].rearrange("(tc p) i -> p tc i", p=128),
                )
                w_n = {}
                for name, dram in (("q", wq_d), ("k", wk_d), ("v", wv_d), ("1", w1_d)):
                    w_n[name] = pro.tile([128, 2 * H], F32, tag=f"wn{name}", bufs=1, name=f"wn{name}")
                    nc.sync.dma_start(
                        w_n[name].rearrange("p (oc i) -> p oc i", oc=2),
                        dram[:].rearrange("(oc p) i -> p oc i", p=128),
                    )

                # weight transposes: W^T[ic][p_i, o] = W[o, ic*128+p_i]
                for name in ("q", "k", "v", "1"):
                    for oc in range(2):
                        for ic in range(2):
                            ps = props.tile([128, 512], F32, tag="pps", name="ps")[:, 0:128]
                            mm.transpose(
                                ps, w_n[name][:, oc * H + ic * 128 : oc * H + (ic + 1) * 128], id_f32
                            )
                            nc.scalar.activation(
                                WT[name][ic][:, oc * 128 : (oc + 1) * 128], ps, AF.Copy
                            )

                # q transpose (f32) -> qT bf16, plus residual row-sums in f32
                qT = [pro.tile([128, T], BF16, tag=f"qtt{ic}", bufs=1, name=f"qtt{ic}") for ic in range(2)]
                res_scr = pro.tile([128, 8], F32, tag="resscr", bufs=1)
                for ic in range(2):
                    for g in range(4):  # groups of 4 token-chunks
                        ps = props.tile([128, 512], F32, tag="pps", name="ps")
                        for j in range(4):
                            tcc = g * 4 + j
                            mm.transpose(
                                ps[:, j * 128 : (j + 1) * 128],
                                q_n[:, tcc * H + ic * 128 : tcc * H + (ic + 1) * 128],
                                id_f32,
                            )
                        (nc.scalar.activation(qT[ic][:, g * 512 : (g + 1) * 512], ps, AF.Copy)
                         if g % 2 == 0 else
                         nc.vector.tensor_copy(qT[ic][:, g * 512 : (g + 1) * 512], ps))
                        nc.vector.reduce_sum(
                            res_scr[:, ic * 4 + g : ic * 4 + g + 1], ps, axis=mybir.AxisListType.X
                        )
                for ic in range(2):
                    for b in range(BL):
                        nc.vector.reduce_sum(
                            resT[ic][:, b : b + 1],
                            res_scr[:, ic * 4 + b * 2 : ic * 4 + (b + 1) * 2],
                            axis=mybir.AxisListType.X,
                        )

                # k, v transposes (bf16)
                kT = [pro.tile([128, T], BF16, tag=f"ktt{ic}", bufs=1, name=f"ktt{ic}") for ic in range(2)]
                vT = [pro.tile([128, T], BF16, tag=f"vtt{ic}", bufs=1, name=f"vtt{ic}") for ic in range(2)]
                for src, dstl in ((k_n, kT), (v_n, vT)):
                    for ic in range(2):
                        for g in range(4):
                            ps = props.tile([128, 1024], BF16, tag="pps", name="ps")[:, 0:512]
                            for j in range(4):
                                tcc = g * 4 + j
                                mm.transpose(
                                    ps[:, j * 128 : (j + 1) * 128],
                                    src[:, tcc * H + ic * 128 : tcc * H + (ic + 1) * 128],
                                    id_bf16,
                                )
                            (nc.scalar.activation(dstl[ic][:, g * 512 : (g + 1) * 512], ps, AF.Copy)
                             if g % 2 == 0 else
                             nc.vector.tensor_copy(dstl[ic][:, g * 512 : (g + 1) * 512], ps))

                # projections: QT[m][p_o, t] = sum_i WqT[i, m*128+p_o] * qT[i, t]
                for m in range(2):
                    for tcol in range(4):
                        psq = props.tile([128, 512], F32, tag="pps", name="psq")
                        psk = props.tile([128, 512], F32, tag="pps", name="psk")
                        for ic in range(2):
                            mm.matmul(
                                psq,
                                WT["q"][ic][:, m * 128 : (m + 1) * 128],
                                qT[ic][:, tcol * 512 : (tcol + 1) * 512],
                                start=(ic == 0),
                                stop=(ic == 1),
                            )
                            mm.matmul(
                                psk,
                                WT["k"][ic][:, m * 128 : (m + 1) * 128],
                                kT[ic][:, tcol * 512 : (tcol + 1) * 512],
                                start=(ic == 0),
                                stop=(ic == 1),
                            )
                        nc.scalar.activation(QT[m][:, tcol * 512 : (tcol + 1) * 512], psq, AF.Copy)
                        nc.vector.tensor_copy(KT[m][:, tcol * 512 : (tcol + 1) * 512], psk)
                # V natural: Vn[p_t, tc*H + o] = sum_i vT[i, t] * WvT[i, o]
                for tcc in range(NTC):
                    psv = props.tile([128, 512], F32, tag="pps", name="psv")[:, 0:256]
                    for ic in range(2):
                        mm.matmul(
                            psv,
                            vT[ic][:, tcc * 128 : (tcc + 1) * 128],
                            WT["v"][ic],
                            start=(ic == 0),
                            stop=(ic == 1),
                        )
                    nc.scalar.activation(Vn[:, tcc * H : (tcc + 1) * H], psv, AF.Copy)

                # BN constants: scaleA = gamma*rsqrt(var+eps); biasB = (b1-mean)*scaleA + beta
                vecs = {}
                for name, dram in (
                    ("b1", b1_d), ("gamma", gamma_d), ("beta", beta_d),
                    ("rmean", rmean_d), ("rvar", rvar_d),
                ):
                    vecs[name] = pro.tile([128, 2], F32, tag=f"vec{name}", bufs=1, name=f"vec{name}")
                    nc.sync.dma_start(
                        vecs[name], dram[:].rearrange("(c p) -> p c", p=128)
                    )
                sqv = pro.tile([128, 2], F32, tag="sqv", bufs=1)
                nc.vector.tensor_scalar_add(sqv, vecs["rvar"], BN_EPS)
                nc.scalar.activation(sqv, sqv, AF.Ln)
                nc.vector.tensor_scalar_mul(sqv, sqv, -0.5)
                nc.scalar.activation(scaleA, sqv, AF.Exp)
                nc.vector.tensor_mul(scaleA, scaleA, vecs["gamma"])
                tmpv = pro.tile([128, 2], F32, tag="tmpv", bufs=1)
                nc.vector.tensor_sub(tmpv, vecs["b1"], vecs["rmean"])
                nc.vector.tensor_mul(tmpv, tmpv, scaleA)
                nc.vector.tensor_add(biasB, tmpv, vecs["beta"])

            # ---------------- main attention loop ----------------
            with tc.tile_pool(name="biasp", bufs=6) as biasp, \
                 tc.tile_pool(name="aup", bufs=40) as aup, \
                 tc.tile_pool(name="aop", bufs=6) as aop, \
                 tc.tile_pool(name="smallp", bufs=2) as smallp, \
                 tc.tile_pool(name="pse", bufs=2, space="PSUM") as pse, \
                 tc.tile_pool(name="pss", bufs=2, space="PSUM") as pss:
                for h in range(NH):
                    hm, hp = h // 4, (h % 4) * 32
                    rs = smallp.tile([128, 2 * 8], F32, tag="rows")
                    au_tiles = {}
                    for tq in range(8):
                        bt = biasp.tile([128, SK], BF16, tag="bias")
                        nc.sync.dma_start(bt, bias_d[h, tq * 128 : (tq + 1) * 128, :])
                        for b in range(BL):
                            pe_t = pse.tile([128, SK], F32, tag="pe")
                            for hf in range(2):
                                mm.matmul(
                                    pe_t[:, hf * 512 : (hf + 1) * 512],
                                    id_bf16,
                                    bt[:, hf * 512 : (hf + 1) * 512],
                                    start=True,
                                    stop=False,
                                )
                            qsl = QT[hm][
                                hp : hp + 32, b * SQ + tq * 128 : b * SQ + (tq + 1) * 128
                            ]
                            ksl = KT[hm][hp : hp + 32, b * SQ : (b + 1) * SQ]
                            for hf in range(2):
                                mm.matmul(
                                    pe_t[:, hf * 512 : (hf + 1) * 512],
                                    qsl,
                                    ksl[:, hf * 512 : (hf + 1) * 512],
                                    start=False,
                                    stop=True,
                                    tile_position=(hp, 0),
                                )
                            au = aup.tile([128, SK], BF16, tag="au")
                            nc.scalar.activation(
                                au, pe_t, AF.Exp,
                                accum_out=rs[:, b * 8 + tq : b * 8 + tq + 1],
                            )
                            au_tiles[(b, tq)] = au
                    u_f = smallp.tile([128, 16], F32, tag="uf")
                    nc.vector.reciprocal(u_f, rs)
                    u_b = smallp.tile([128, 16], BF16, tag="ub")
                    nc.vector.tensor_copy(u_b, u_f)
                    for b in range(BL):
                        s_ps = pss.tile([1, SK], F32, tag="sps")
                        for tq in range(8):
                            au = au_tiles[(b, tq)]
                            ao = aop.tile([128, SK], F32, tag="ao")
                            nc.vector.tensor_scalar_mul(
                                ao, au, u_f[:, b * 8 + tq : b * 8 + tq + 1]
                            )
                            for hf in range(2):
                                mm.matmul(
                                    s_ps[0:1, hf * 512 : (hf + 1) * 512],
                                    u_b[:, b * 8 + tq : b * 8 + tq + 1],
                                    au[:, hf * 512 : (hf + 1) * 512],
                                    start=(tq == 0),
                                    stop=(tq == 7),
                                )
                            nc.scalar.dma_start(
                                attn_d[b, h, tq * 128 : (tq + 1) * 128, :], ao
                            )
                        s_sb = smallp.tile([1, SK], F32, tag="ssb", name="s_sb")
                        nc.vector.tensor_copy(s_sb, s_ps)
                        nc.sync.dma_start(S_all[b * NH + h : b * NH + h + 1, :], s_sb)

            # ---------------- tail ----------------
            with tc.tile_pool(name="tailp", bufs=2) as tailp, \
                 tc.tile_pool(name="tailps", bufs=1, space="PSUM") as tailps:
                # transpose S_all -> sT_all[p_tk, c*16 + (b*8+h)]
                for c in range(8):
                    ps = tailps.tile([128, BL * NH], F32, tag="stp", bufs=2, name="ps")
                    mm.transpose(
                        ps, S_all[:, c * 128 : (c + 1) * 128], id_f32[0 : BL * NH, 0 : BL * NH]
                    )
                    nc.scalar.activation(sT_all[:, c * 16 : (c + 1) * 16], ps, AF.Copy)

                out_sb = []
                for b in range(BL):
                    xs_ps = tailps.tile([128, 2], F32, tag=f"xs{b}")
                    for h in range(NH):
                        for c in range(8):
                            mm.matmul(
                                xs_ps[(h % 4) * 32 : (h % 4) * 32 + 32, h // 4 : h // 4 + 1],
                                Vn[:, (b * 8 + c) * H + h * HD : (b * 8 + c) * H + (h + 1) * HD],
                                sT_all[:, c * 16 + b * NH + h : c * 16 + b * NH + h + 1],
                                start=(c == 0),
                                stop=(c == 7),
                                tile_position=(0, (h % 4) * 32),
                            )
                    xs_sb = tailp.tile([128, 2], BF16, tag=f"xsb{b}")
                    nc.vector.tensor_copy(xs_sb, xs_ps)
                    h_ps = tailps.tile([128, 2], F32, tag=f"hps{b}")
                    for jc in range(2):
                        for ic in range(2):
                            mm.matmul(
                                h_ps[:, jc : jc + 1],
                                WT["1"][ic][:, jc * 128 : (jc + 1) * 128],
                                xs_sb[:, ic : ic + 1],
                                start=(ic == 0),
                                stop=(ic == 1),
                            )
                    z_t = tailp.tile([128, 2], F32, tag=f"z{b}", name=f"z{b}")
                    e_t = tailp.tile([128, 2], F32, tag=f"e{b}", name=f"e{b}")
                    for jc in range(2):
                        nc.vector.tensor_scalar(
                            z_t[:, jc : jc + 1],
                            h_ps[:, jc : jc + 1],
                            scaleA[:, jc : jc + 1],
                            biasB[:, jc : jc + 1],
                            op0=mybir.AluOpType.mult,
                            op1=mybir.AluOpType.add,
                        )
                    nc.scalar.activation(e_t, z_t, AF.Exp)
                    u_t = tailp.tile([128, 2], F32, tag=f"u{b}", name=f"u{b}")
                    nc.vector.tensor_scalar_add(u_t, e_t, 1.0)
                    w_t = tailp.tile([128, 2], F32, tag=f"w{b}", name=f"w{b}")
                    nc.vector.tensor_mul(w_t, u_t, u_t)
                    a_t = tailp.tile([128, 2], F32, tag=f"a{b}", name=f"a{b}")
                    nc.vector.tensor_scalar_add(a_t, w_t, -1.0)
                    b_t = tailp.tile([128, 2], F32, tag=f"bb{b}", name=f"bb{b}")
                    nc.vector.tensor_scalar_add(b_t, w_t, 1.0)
                    r_t = tailp.tile([128, 2], F32, tag=f"r{b}", name=f"r{b}")
                    nc.vector.reciprocal(r_t, b_t)
                    t_t = tailp.tile([128, 2], F32, tag=f"t{b}", name=f"t{b}")
                    nc.vector.tensor_mul(t_t, a_t, r_t)
                    msh = tailp.tile([128, 2], F32, tag=f"msh{b}", name=f"msh{b}")
                    nc.vector.tensor_mul(msh, z_t, t_t)
                    ot = tailp.tile([128, 2], F32, tag=f"ot{b}")
                    for jc in range(2):
                        nc.vector.tensor_add(
                            ot[:, jc : jc + 1], msh[:, jc : jc + 1], resT[jc][:, b : b + 1]
                        )
                    out_sb.append(ot)
                    nc.sync.dma_start(
                        out_d[b, :].rearrange("(c p) -> p c", p=128), ot
                    )

    nc.finalize()
    return nc


_CACHE = {}


def _get_nc():
    if "nc" not in _CACHE:
        _CACHE["nc"] = build()
    return _CACHE["nc"]


def kernel(q, k, v, attn_bias, W_Q, W_K, W_V, W1, b1, gamma, beta, run_mean, run_var):
    import ml_dtypes

    nc = _get_nc()

    f32 = lambda x: np.ascontiguousarray(np.asarray(x, dtype=np.float32))
    q = f32(q).reshape(B, SQ, H)
    k_b = np.asarray(k, dtype=np.float32).astype(ml_dtypes.bfloat16).reshape(B, SK, H)
    v_b = np.asarray(v, dtype=np.float32).astype(ml_dtypes.bfloat16).reshape(B, SK, H)
    bias_b = np.ascontiguousarray(
        np.asarray(attn_bias, dtype=np.float32).astype(ml_dtypes.bfloat16).reshape(NH, SQ, SK)
    )
    ident = np.eye(128, dtype=np.float32)
    shared = {
        "bias": bias_b,
        "wq": f32(W_Q) * np.float32(INV_SQRT_HD), "wk": f32(W_K), "wv": f32(W_V), "w1": f32(W1),
        "b1": f32(b1), "gamma": f32(gamma), "beta": f32(beta),
        "rmean": f32(run_mean), "rvar": f32(run_var),
        "ident": ident,
    }
    in_maps = []
    for c in range(N_CORES):
        in_maps.append(
            dict(
                shared,
                q=np.ascontiguousarray(q[BL * c : BL * (c + 1)].reshape(T, H)),
                k=np.ascontiguousarray(k_b[BL * c : BL * (c + 1)].reshape(T, H)),
                v=np.ascontiguousarray(v_b[BL * c : BL * (c + 1)].reshape(T, H)),
            )
        )

    trace = os.environ.get("BASS_KERNEL_TRACE") == "1"
    if trace:
        sys.path.insert(0, os.path.dirname(os.path.abspath(__file__)))
        try:
            import ntff_hook

            ntff_hook.install()
        except Exception as e:  # profiling is best-effort
            print("ntff hook install failed:", e)
            trace = False

    res = run_bass_kernel_spmd(nc, in_maps, core_ids=list(range(N_CORES)), trace=trace)
    if trace:
        print(f"HW exec time: {res.exec_time_ns} ns")
        _CACHE["last_result"] = res

    attn = np.concatenate([res.results[c]["attn"] for c in range(N_CORES)], axis=0)
    out = np.concatenate([res.results[c]["out"] for c in range(N_CORES)], axis=0)
    return out.astype(np.float32), attn.astype(np.float32)


# revision 11
# speedup vs baseline: 1.2702x; 1.2702x over previous
"""Trainium2 Bass kernel for nn_AttentionBlock (B=16, S=1024, H=256, NH=8).

Strategy: data-parallel over batch — each of 8 NeuronCores handles 2 batches
end-to-end (no collectives). Per core:
  - project q,k,v -> Q^T,K^T (bf16, [out_feat, token] layout) and V (natural)
  - energy = Q_h K_h^T / sqrt(hd) + bias  via PE matmuls; bias is injected
    into PSUM with an identity-weight matmul so the QK matmul accumulates
    on top of it
  - softmax without max-subtraction (logits are ~N(0, 0.15), safe):
    exp on ScalarE with fused row-sum (accum_out), reciprocal + per-row
    scale on VectorE
  - attention matrix (the big 512 MB output) is DMAed out tile by tile
  - the MLP tail only needs x.sum(axis=1) = colsum(attention) @ V, so the
    full attention@V einsum is never materialized: colsums come from a
    ones/recip-weighted PE matmul over the unnormalized exp tiles
  - tail: xs@W1.T + BN(eval) + Mish + residual, all tiny, on-chip
"""

import os
import sys

sys.path.insert(0, "/opt/trn_rl_repo")

import numpy as np

import concourse.bass as bass
import concourse.bacc as bacc
import concourse.mybir as mybir
import concourse.tile as tile
from concourse.bass_utils import run_bass_kernel_spmd

N_CORES = 8
B, SQ, SK, H, NH = 16, 1024, 1024, 256, 8
HD = H // NH  # 32
BL = B // N_CORES  # 2 batches per core
T = BL * SQ  # 2048 local tokens
NTC = T // 128  # 16 token chunks
INV_SQRT_HD = float(1.0 / np.sqrt(HD))
BN_EPS = 1e-5

F32 = mybir.dt.float32
BF16 = mybir.dt.bfloat16
AF = mybir.ActivationFunctionType


def build():
    nc = bacc.Bacc()

    q_d = nc.dram_tensor("q", [T, H], F32, kind="ExternalInput")
    k_d = nc.dram_tensor("k", [T, H], BF16, kind="ExternalInput")
    v_d = nc.dram_tensor("v", [T, H], BF16, kind="ExternalInput")
    bias_d = nc.dram_tensor("bias", [NH, SQ, SK], BF16, kind="ExternalInput")
    wq_d = nc.dram_tensor("wq", [H, H], F32, kind="ExternalInput")
    wk_d = nc.dram_tensor("wk", [H, H], F32, kind="ExternalInput")
    wv_d = nc.dram_tensor("wv", [H, H], F32, kind="ExternalInput")
    w1_d = nc.dram_tensor("w1", [H, H], F32, kind="ExternalInput")
    b1_d = nc.dram_tensor("b1", [H], F32, kind="ExternalInput")
    gamma_d = nc.dram_tensor("gamma", [H], F32, kind="ExternalInput")
    beta_d = nc.dram_tensor("beta", [H], F32, kind="ExternalInput")
    rmean_d = nc.dram_tensor("rmean", [H], F32, kind="ExternalInput")
    rvar_d = nc.dram_tensor("rvar", [H], F32, kind="ExternalInput")
    ident_d = nc.dram_tensor("ident", [128, 128], F32, kind="ExternalInput")

    attn_d = nc.dram_tensor("attn", [BL, NH, SQ, SK], BF16, kind="ExternalOutput")
    out_d = nc.dram_tensor("out", [BL, H], F32, kind="ExternalOutput")

    mm = nc.tensor

    with tile.TileContext(nc) as tc:
        with tc.tile_pool(name="persist", bufs=1) as persist:
            # --- constants ---
            id_f32 = persist.tile([128, 128], F32, tag="idf")
            nc.sync.dma_start(id_f32, ident_d[:])
            id_bf16 = persist.tile([128, 128], BF16, tag="idb")
            nc.scalar.activation(id_bf16, id_f32, AF.Copy)

            # persistent activations
            QT = [persist.tile([128, T], BF16, tag=f"qt{m}", name=f"qt{m}") for m in range(2)]
            KT = [persist.tile([128, T], BF16, tag=f"kt{m}", name=f"kt{m}") for m in range(2)]
            Vn = persist.tile([128, NTC * H], BF16, tag="vn")
            WT = {
                w: [persist.tile([128, H], BF16, tag=f"w{w}{ic}", name=f"w{w}{ic}") for ic in range(2)]
                for w in ("q", "k", "v", "1")
            }
            resT = [persist.tile([128, BL], F32, tag=f"res{ic}", name=f"res{ic}") for ic in range(2)]
            S_all = persist.tile([BL * NH, SK], F32, tag="sall")
            sT_all = persist.tile([128, NTC * BL * NH // 2], BF16, tag="stall")
            scaleA = persist.tile([128, 2], F32, tag="scla")
            biasB = persist.tile([128, 2], F32, tag="bnb")

            # ---------------- prologue ----------------
            with tc.tile_pool(name="pro", bufs=2) as pro, \
                 tc.tile_pool(name="props", bufs=4, space="PSUM") as props:
                # natural-layout loads
                q_n = pro.tile([128, NTC * H], F32, tag="qn", bufs=1)
                nc.sync.dma_start(
                    q_n.rearrange("p (tc i) -> p tc i", tc=NTC),
                    q_d[:].rearrange("(tc p) i -> p tc i", p=128),
                )
                k_n = pro.tile([128, NTC * H], BF16, tag="kn", bufs=1)
                nc.sync.dma_start(
                    k_n.rearrange("p (tc i) -> p tc i", tc=NTC),
                    k_d[:].rearrange("(tc p) i -> p tc i", p=128),
                )
                v_n = pro.tile([128, NTC * H], BF16, tag="vnat", bufs=1)
                nc.sync.dma_start(
                    v_n.rearrange("p (tc i) -> p tc i", tc=NTC),
                    v_d[:].rearrange("(tc p) i -> p tc i", p=128),
                )
                w_n = {}
                for name, dram in (("q", wq_d), ("k", wk_d), ("v", wv_d), ("1", w1_d)):
                    w_n[name] = pro.tile([128, 2 * H], F32, tag=f"wn{name}", bufs=1, name=f"wn{name}")
                    nc.sync.dma_start(
                        w_n[name].rearrange("p (oc i) -> p oc i", oc=2),
                        dram[:].rearrange("(oc p) i -> p oc i", p=128),
                    )

                # weight transposes: W^T[ic][p_i, o] = W[o, ic*128+p_i]
                for name in ("q", "k", "v", "1"):
                    for oc in range(2):
                        for ic in range(2):
                            ps = props.tile([128, 512], F32, tag="pps", name="ps")[:, 0:128]
                            mm.transpose(
                                ps, w_n[name][:, oc * H + ic * 128 : oc * H + (ic + 1) * 128], id_f32
                            )
                            nc.scalar.activation(
                                WT[name][ic][:, oc * 128 : (oc + 1) * 128], ps, AF.Copy
                            )

                # q transpose (f32) -> qT bf16, plus residual row-sums in f32
                qT = [pro.tile([128, T], BF16, tag=f"qtt{ic}", bufs=1, name=f"qtt{ic}") for ic in range(2)]
                res_scr = pro.tile([128, 8], F32, tag="resscr", bufs=1)
                for ic in range(2):
                    for g in range(4):  # groups of 4 token-chunks
                        ps = props.tile([128, 512], F32, tag="pps", name="ps")
                        for j in range(4):
                            tcc = g * 4 + j
                            mm.transpose(
                                ps[:, j * 128 : (j + 1) * 128],
                                q_n[:, tcc * H + ic * 128 : tcc * H + (ic + 1) * 128],
                                id_f32,
                            )
                        (nc.scalar.activation(qT[ic][:, g * 512 : (g + 1) * 512], ps, AF.Copy)
                         if g % 2 == 0 else
                         nc.vector.tensor_copy(qT[ic][:, g * 512 : (g + 1) * 512], ps))
                        nc.vector.reduce_sum(
                            res_scr[:, ic * 4 + g : ic * 4 + g + 1], ps, axis=mybir.AxisListType.X
                        )
                for ic in range(2):
                    for b in range(BL):
                        nc.vector.reduce_sum(
                            resT[ic][:, b : b + 1],
                            res_scr[:, ic * 4 + b * 2 : ic * 4 + (b + 1) * 2],
                            axis=mybir.AxisListType.X,
                        )

                # k, v transposes (bf16)
                kT = [pro.tile([128, T], BF16, tag=f"ktt{ic}", bufs=1, name=f"ktt{ic}") for ic in range(2)]
                vT = [pro.tile([128, T], BF16, tag=f"vtt{ic}", bufs=1, name=f"vtt{ic}") for ic in range(2)]
                for src, dstl in ((k_n, kT), (v_n, vT)):
                    for ic in range(2):
                        for g in range(4):
                            ps = props.tile([128, 1024], BF16, tag="pps", name="ps")[:, 0:512]
                            for j in range(4):
                                tcc = g * 4 + j
                                mm.transpose(
                                    ps[:, j * 128 : (j + 1) * 128],
                                    src[:, tcc * H + ic * 128 : tcc * H + (ic + 1) * 128],
                                    id_bf16,
                                )
                            (nc.scalar.activation(dstl[ic][:, g * 512 : (g + 1) * 512], ps, AF.Copy)
                             if g % 2 == 0 else
                             nc.vector.tensor_copy(dstl[ic][:, g * 512 : (g + 1) * 512], ps))

                # projections: QT[m][p_o, t] = sum_i WqT[i, m*128+p_o] * qT[i, t]
                for m in range(2):
                    for tcol in range(4):
                        psq = props.tile([128, 512], F32, tag="pps", name="psq")
                        psk = props.tile([128, 512], F32, tag="pps", name="psk")
                        for ic in range(2):
                            mm.matmul(
                                psq,
                                WT["q"][ic][:, m * 128 : (m + 1) * 128],
                                qT[ic][:, tcol * 512 : (tcol + 1) * 512],
                                start=(ic == 0),
                                stop=(ic == 1),
                            )
                            mm.matmul(
                                psk,
                                WT["k"][ic][:, m * 128 : (m + 1) * 128],
                                kT[ic][:, tcol * 512 : (tcol + 1) * 512],
                                start=(ic == 0),
                                stop=(ic == 1),
                            )
                        nc.scalar.activation(QT[m][:, tcol * 512 : (tcol + 1) * 512], psq, AF.Copy)
                        nc.vector.tensor_copy(KT[m][:, tcol * 512 : (tcol + 1) * 512], psk)
                # V natural: Vn[p_t, tc*H + o] = sum_i vT[i, t] * WvT[i, o]
                for tcc in range(NTC):
                    psv = props.tile([128, 512], F32, tag="pps", name="psv")[:, 0:256]
                    for ic in range(2):
                        mm.matmul(
                            psv,
                            vT[ic][:, tcc * 128 : (tcc + 1) * 128],
                            WT["v"][ic],
                            start=(ic == 0),
                            stop=(ic == 1),
                        )
                    nc.scalar.activation(Vn[:, tcc * H : (tcc + 1) * H], psv, AF.Copy)

                # BN constants: scaleA = gamma*rsqrt(var+eps); biasB = (b1-mean)*scaleA + beta
                vecs = {}
                for name, dram in (
                    ("b1", b1_d), ("gamma", gamma_d), ("beta", beta_d),
                    ("rmean", rmean_d), ("rvar", rvar_d),
                ):
                    vecs[name] = pro.tile([128, 2], F32, tag=f"vec{name}", bufs=1, name=f"vec{name}")
                    nc.sync.dma_start(
                        vecs[name], dram[:].rearrange("(c p) -> p c", p=128)
                    )
                sqv = pro.tile([128, 2], F32, tag="sqv", bufs=1)
                nc.vector.tensor_scalar_add(sqv, vecs["rvar"], BN_EPS)
                nc.scalar.activation(sqv, sqv, AF.Ln)
                nc.vector.tensor_scalar_mul(sqv, sqv, -0.5)
                nc.scalar.activation(scaleA, sqv, AF.Exp)
                nc.vector.tensor_mul(scaleA, scaleA, vecs["gamma"])
                tmpv = pro.tile([128, 2], F32, tag="tmpv", bufs=1)
                nc.vector.tensor_sub(tmpv, vecs["b1"], vecs["rmean"])
                nc.vector.tensor_mul(tmpv, tmpv, scaleA)
                nc.vector.tensor_add(biasB, tmpv, vecs["beta"])

            # ---------------- main attention loop ----------------
            with tc.tile_pool(name="biasp", bufs=6) as biasp, \
                 tc.tile_pool(name="aup", bufs=40) as aup, \
                 tc.tile_pool(name="aop", bufs=6) as aop, \
                 tc.tile_pool(name="smallp", bufs=2) as smallp, \
                 tc.tile_pool(name="pse", bufs=2, space="PSUM") as pse, \
                 tc.tile_pool(name="pss", bufs=2, space="PSUM") as pss:
                state = {}

                def phase_a(h):
                    hm, hp = h // 4, (h % 4) * 32
                    rs = smallp.tile([128, 2 * 8], F32, tag="rows", name="rs")
                    au_tiles = {}
                    for tq in range(8):
                        bt = biasp.tile([128, SK], BF16, tag="bias", name="bt")
                        nc.sync.dma_start(bt, bias_d[h, tq * 128 : (tq + 1) * 128, :])
                        for b in range(BL):
                            pe_t = pse.tile([128, SK], F32, tag="pe", name="pe_t")
                            for hf in range(2):
                                mm.matmul(
                                    pe_t[:, hf * 512 : (hf + 1) * 512],
                                    id_bf16,
                                    bt[:, hf * 512 : (hf + 1) * 512],
                                    start=True,
                                    stop=False,
                                )
                            qsl = QT[hm][
                                hp : hp + 32, b * SQ + tq * 128 : b * SQ + (tq + 1) * 128
                            ]
                            ksl = KT[hm][hp : hp + 32, b * SQ : (b + 1) * SQ]
                            for hf in range(2):
                                mm.matmul(
                                    pe_t[:, hf * 512 : (hf + 1) * 512],
                                    qsl,
                                    ksl[:, hf * 512 : (hf + 1) * 512],
                                    start=False,
                                    stop=True,
                                    tile_position=(hp, 0),
                                )
                            au = aup.tile([128, SK], BF16, tag="au", name="au")
                            nc.scalar.activation(
                                au, pe_t, AF.Exp,
                                accum_out=rs[:, b * 8 + tq : b * 8 + tq + 1],
                            )
                            au_tiles[(b, tq)] = au
                    state[h] = (rs, au_tiles)

                def phase_b(h):
                    rs, au_tiles = state.pop(h)
                    u_f = smallp.tile([128, 16], F32, tag="uf", name="u_f")
                    nc.vector.reciprocal(u_f, rs)
                    u_b = smallp.tile([128, 16], BF16, tag="ub", name="u_b")
                    nc.vector.tensor_copy(u_b, u_f)
                    for b in range(BL):
                        s_ps = pss.tile([1, SK], F32, tag="sps", name="s_ps")
                        for tq in range(8):
                            au = au_tiles[(b, tq)]
                            ao = aop.tile([128, SK], BF16, tag="ao", name="ao")
                            nc.vector.tensor_scalar_mul(
                                ao, au, u_f[:, b * 8 + tq : b * 8 + tq + 1]
                            )
                            for hf in range(2):
                                mm.matmul(
                                    s_ps[0:1, hf * 512 : (hf + 1) * 512],
                                    u_b[:, b * 8 + tq : b * 8 + tq + 1],
                                    au[:, hf * 512 : (hf + 1) * 512],
                                    start=(tq == 0),
                                    stop=(tq == 7),
                                )
                            nc.scalar.dma_start(
                                attn_d[b, h, tq * 128 : (tq + 1) * 128, :], ao
                            )
                        s_sb = smallp.tile([1, SK], F32, tag="ssb", name="s_sb")
                        nc.vector.tensor_copy(s_sb, s_ps)
                        nc.sync.dma_start(S_all[b * NH + h : b * NH + h + 1, :], s_sb)

                for h in range(NH):
                    phase_a(h)
                    if h > 0:
                        phase_b(h - 1)
                phase_b(NH - 1)

            # ---------------- tail ----------------
            with tc.tile_pool(name="tailp", bufs=2) as tailp, \
                 tc.tile_pool(name="tailps", bufs=1, space="PSUM") as tailps:
                # transpose S_all -> sT_all[p_tk, c*16 + (b*8+h)]
                for c in range(8):
                    ps = tailps.tile([128, BL * NH], F32, tag="stp", bufs=2, name="ps")
                    mm.transpose(
                        ps, S_all[:, c * 128 : (c + 1) * 128], id_f32[0 : BL * NH, 0 : BL * NH]
                    )
                    nc.scalar.activation(sT_all[:, c * 16 : (c + 1) * 16], ps, AF.Copy)

                out_sb = []
                for b in range(BL):
                    xs_ps = tailps.tile([128, 2], F32, tag=f"xs{b}")
                    for h in range(NH):
                        for c in range(8):
                            mm.matmul(
                                xs_ps[(h % 4) * 32 : (h % 4) * 32 + 32, h // 4 : h // 4 + 1],
                                Vn[:, (b * 8 + c) * H + h * HD : (b * 8 + c) * H + (h + 1) * HD],
                                sT_all[:, c * 16 + b * NH + h : c * 16 + b * NH + h + 1],
                                start=(c == 0),
                                stop=(c == 7),
                                tile_position=(0, (h % 4) * 32),
                            )
                    xs_sb = tailp.tile([128, 2], BF16, tag=f"xsb{b}")
                    nc.vector.tensor_copy(xs_sb, xs_ps)
                    h_ps = tailps.tile([128, 2], F32, tag=f"hps{b}")
                    for jc in range(2):
                        for ic in range(2):
                            mm.matmul(
                                h_ps[:, jc : jc + 1],
                                WT["1"][ic][:, jc * 128 : (jc + 1) * 128],
                                xs_sb[:, ic : ic + 1],
                                start=(ic == 0),
                                stop=(ic == 1),
                            )
                    z_t = tailp.tile([128, 2], F32, tag=f"z{b}", name=f"z{b}")
                    e_t = tailp.tile([128, 2], F32, tag=f"e{b}", name=f"e{b}")
                    for jc in range(2):
                        nc.vector.tensor_scalar(
                            z_t[:, jc : jc + 1],
                            h_ps[:, jc : jc + 1],
                            scaleA[:, jc : jc + 1],
                            biasB[:, jc : jc + 1],
                            op0=mybir.AluOpType.mult,
                            op1=mybir.AluOpType.add,
                        )
                    nc.scalar.activation(e_t, z_t, AF.Exp)
                    u_t = tailp.tile([128, 2], F32, tag=f"u{b}", name=f"u{b}")
                    nc.vector.tensor_scalar_add(u_t, e_t, 1.0)
                    w_t = tailp.tile([128, 2], F32, tag=f"w{b}", name=f"w{b}")
                    nc.vector.tensor_mul(w_t, u_t, u_t)
                    a_t = tailp.tile([128, 2], F32, tag=f"a{b}", name=f"a{b}")
                    nc.vector.tensor_scalar_add(a_t, w_t, -1.0)
                    b_t = tailp.tile([128, 2], F32, tag=f"bb{b}", name=f"bb{b}")
                    nc.vector.tensor_scalar_add(b_t, w_t, 1.0)
                    r_t = tailp.tile([128, 2], F32, tag=f"r{b}", name=f"r{b}")
                    nc.vector.reciprocal(r_t, b_t)
                    t_t = tailp.tile([128, 2], F32, tag=f"t{b}", name=f"t{b}")
                    nc.vector.tensor_mul(t_t, a_t, r_t)
                    msh = tailp.tile([128, 2], F32, tag=f"msh{b}", name=f"msh{b}")
                    nc.vector.tensor_mul(msh, z_t, t_t)
                    ot = tailp.tile([128, 2], F32, tag=f"ot{b}")
                    for jc in range(2):
                        nc.vector.tensor_add(
                            ot[:, jc : jc + 1], msh[:, jc : jc + 1], resT[jc][:, b : b + 1]
                        )
                    out_sb.append(ot)
                    nc.sync.dma_start(
                        out_d[b, :].rearrange("(c p) -> p c", p=128), ot
                    )

    nc.finalize()
    return nc


_CACHE = {}


def _get_nc():
    if "nc" not in _CACHE:
        _CACHE["nc"] = build()
    return _CACHE["nc"]


def kernel(q, k, v, attn_bias, W_Q, W_K, W_V, W1, b1, gamma, beta, run_mean, run_var):
    import ml_dtypes

    nc = _get_nc()

    f32 = lambda x: np.ascontiguousarray(np.asarray(x, dtype=np.float32))
    q = f32(q).reshape(B, SQ, H)
    k_b = np.asarray(k, dtype=np.float32).astype(ml_dtypes.bfloat16).reshape(B, SK, H)
    v_b = np.asarray(v, dtype=np.float32).astype(ml_dtypes.bfloat16).reshape(B, SK, H)
    bias_b = np.ascontiguousarray(
        np.asarray(attn_bias, dtype=np.float32).astype(ml_dtypes.bfloat16).reshape(NH, SQ, SK)
    )
    ident = np.eye(128, dtype=np.float32)
    shared = {
        "bias": bias_b,
        "wq": f32(W_Q) * np.float32(INV_SQRT_HD), "wk": f32(W_K), "wv": f32(W_V), "w1": f32(W1),
        "b1": f32(b1), "gamma": f32(gamma), "beta": f32(beta),
        "rmean": f32(run_mean), "rvar": f32(run_var),
        "ident": ident,
    }
    in_maps = []
    for c in range(N_CORES):
        in_maps.append(
            dict(
                shared,
                q=np.ascontiguousarray(q[BL * c : BL * (c + 1)].reshape(T, H)),
                k=np.ascontiguousarray(k_b[BL * c : BL * (c + 1)].reshape(T, H)),
                v=np.ascontiguousarray(v_b[BL * c : BL * (c + 1)].reshape(T, H)),
            )
        )

    trace = os.environ.get("BASS_KERNEL_TRACE") == "1"
    if trace:
        sys.path.insert(0, os.path.dirname(os.path.abspath(__file__)))
        try:
            import ntff_hook

            ntff_hook.install()
        except Exception as e:  # profiling is best-effort
            print("ntff hook install failed:", e)
            trace = False

    res = run_bass_kernel_spmd(nc, in_maps, core_ids=list(range(N_CORES)), trace=trace)
    if trace:
        print(f"HW exec time: {res.exec_time_ns} ns")
        _CACHE["last_result"] = res

    attn = np.concatenate([res.results[c]["attn"] for c in range(N_CORES)], axis=0)
    out = np.concatenate([res.results[c]["out"] for c in range(N_CORES)], axis=0)
    return out.astype(np.float32), attn.astype(np.float32)
